# revision 6
# baseline (speedup 1.0000x reference)
"""KAN 3x3 convolution on Trainium2, data-parallel over batch on 8 NeuronCores.

Math: for x in [0,1), each tap's KAN function g_p(t) = sum_c sw[p,c]*B_c(t) +
bw[p]*silu(t) is a cubic spline with interior knots only at 0.2 and 0.6, so
  g_p(t) = a0_p + a1_p*t + a2_p*t^2 + a3_p*t^3
           + beta_p*relu(t-0.2)^3 + gamma_p*relu(t-0.6)^3 + bw_p*silu(t)
and the whole layer is a 6-feature 3x3 convolution plus a constant:
  out = sum_f conv3x3(Phi_f, K_f) + K0,  Phi = {x, x^2, x^3, r6^3, r7^3, silu}.
The 3x3 convs run on the TensorEngine as banded matmuls over 128-row blocks
(vertical taps live in the band, horizontal taps as 3 column-shifted rhs views,
features/taps accumulated in PSUM).
"""
import numpy as np

try:
    from concourse import bass, bacc, mybir, tile
    from concourse.bass_utils import run_bass_kernel_spmd
except ImportError:  # fallback for environments without PYTHONPATH set
    import sys
    for p in ("/opt/trn_rl_repo", "/root/.axon_site/_ro/trn_rl_repo"):
        if p not in sys.path:
            sys.path.insert(0, p)
    from concourse import bass, bacc, mybir, tile
    from concourse.bass_utils import run_bass_kernel_spmd

NCORES = 8
B, H, W = 32, 256, 256
BPC = B // NCORES           # 4 images per core
HO, WO = H - 2, W - 2       # 254 x 254
GRID_SIZE, SPLINE_ORDER = 5, 3
GRID_LO, GRID_HI = -1.0, 1.0


# ---------------- host-side coefficient derivation (float64) ----------------

def _b_splines_np(xe, grid):
    bases = ((xe >= grid[:-1]) & (xe < grid[1:])).astype(xe.dtype)
    for k in range(1, SPLINE_ORDER + 1):
        left = (xe - grid[: -(k + 1)]) / (grid[k:-1] - grid[: -(k + 1)])
        right = (grid[k + 1:] - xe) / (grid[k + 1:] - grid[1:-k])
        bases = left * bases[..., :-1] + right * bases[..., 1:]
    return bases


def _derive_coeffs(base_weight, spline_weight):
    """g_p(t) on [0,1) -> (poly (9,4), beta (9,), gamma (9,), bw (9,)) float64."""
    h = (GRID_HI - GRID_LO) / GRID_SIZE
    grid = np.arange(-SPLINE_ORDER, GRID_SIZE + SPLINE_ORDER + 1,
                     dtype=np.float64) * h + GRID_LO
    sw = np.asarray(spline_weight, np.float64).reshape(9, GRID_SIZE + SPLINE_ORDER)
    cell_poly = []
    for lo, hi in [(0.0, 0.2), (0.2, 0.6), (0.6, 1.0)]:
        ts = np.linspace(lo + (hi - lo) * 0.1, hi - (hi - lo) * 0.1, 4)
        V = np.vander(ts, 4, increasing=True)
        Bs = _b_splines_np(ts[:, None], grid)
        cell_poly.append(np.linalg.solve(V, Bs).T)     # (8 bases, 4 coeffs)
    S = [sw @ cp for cp in cell_poly]                  # per-cell (9,4)
    poly = S[0]
    beta = S[1][:, 3] - S[0][:, 3]
    gamma = S[2][:, 3] - S[1][:, 3]
    bw = np.asarray(base_weight, np.float64).reshape(9)
    return poly, beta, gamma, bw


def _build_weight_inputs(base_weight, spline_weight):
    poly, beta, gamma, bw = _derive_coeffs(base_weight, spline_weight)
    # feature kernels, each (3,3) with p = di*3 + dj
    Wk = np.stack([
        poly[:, 1].reshape(3, 3), poly[:, 2].reshape(3, 3),
        poly[:, 3].reshape(3, 3), beta.reshape(3, 3),
        gamma.reshape(3, 3), bw.reshape(3, 3),
    ])  # (6, 3, 3)
    k0 = poly[:, 0].sum()
    # banded stationary matrices: idx = blk*18 + f*3 + dj, each (128,128)
    # blk0: band[k,m] = W[f, k-m,   dj]  (out rows 0..127, rows 126/127 partial)
    # blk1: band[k,m] = W[f, k-m+2, dj]  (out rows 126..253, rows 0/1 partial)
    k_idx = np.arange(128)[:, None]
    m_idx = np.arange(128)[None, :]
    bands = np.zeros((36, 128, 128), np.float64)
    for f in range(6):
        for dj in range(3):
            for blk, off in ((0, 0), (1, 2)):
                di = k_idx - m_idx + off
                msk = (di >= 0) & (di <= 2)
                bands[blk * 18 + f * 3 + dj][msk] = Wk[f, :, dj][di[msk]]
    consts = np.zeros((128, 4), np.float32)
    consts[:, 0] = k0
    consts[:, 1] = -0.2
    consts[:, 2] = -0.6
    consts[:, 3] = k0
    consts[0:2, 3] = 0.0   # seam rows already carry K0 from block A's export
    return bands.astype(np.float32), consts


# ---------------- device program ----------------

_PROG = None


def _build_program():
    f32 = mybir.dt.float32
    AF = mybir.ActivationFunctionType
    nc = bacc.Bacc("TRN2", target_bir_lowering=False, debug=False,
                   num_devices=NCORES)
    x_d = nc.dram_tensor("x", [BPC, H, W], f32, kind="ExternalInput")
    bands_d = nc.dram_tensor("bands", [36, 128, 128], f32, kind="ExternalInput")
    k0_d = nc.dram_tensor("k0", [128, 4], f32, kind="ExternalInput")
    out_d = nc.dram_tensor("out", [BPC, HO, WO], f32, kind="ExternalOutput")

    with tile.TileContext(nc) as tc:
        with tc.tile_pool(name="main", bufs=1) as pool, \
             tc.tile_pool(name="ps", bufs=1, space="PSUM") as pp:
            bands_t = pool.tile([128, 36 * 128], f32)
            for i in range(36):
                nc.sync.dma_start(bands_t[:, i * 128:(i + 1) * 128], bands_d[i])
            k0_t = pool.tile([128, 4], f32)
            nc.sync.dma_start(k0_t[:], k0_d[:])

            # x layout: [128 rows, img*512 + blk*256 + col]
            xt = pool.tile([128, BPC * 512], f32)
            for img in range(BPC):
                for blk in range(2):
                    nc.sync.dma_start(
                        xt[:, (img * 2 + blk) * 256:(img * 2 + blk + 1) * 256],
                        x_d[img, blk * 128:(blk + 1) * 128, :])

            # features: x, x^2, x^3, relu(x-.2)^3, relu(x-.6)^3, silu(x)
            t2 = pool.tile([128, BPC * 512], f32)
            t3 = pool.tile([128, BPC * 512], f32)
            r6 = pool.tile([128, BPC * 512], f32)
            r6s = pool.tile([128, BPC * 512], f32)
            u = pool.tile([128, BPC * 512], f32)
            r7 = pool.tile([128, BPC * 512], f32)
            r7s = pool.tile([128, BPC * 512], f32)
            v = pool.tile([128, BPC * 512], f32)
            sg = pool.tile([128, BPC * 512], f32)
            sl = pool.tile([128, BPC * 512], f32)
            nc.scalar.activation(t2[:], xt[:], AF.Square)
            nc.vector.tensor_mul(t3[:], t2[:], xt[:])
            nc.scalar.activation(r6[:], xt[:], AF.Relu, bias=k0_t[:, 1:2])
            nc.scalar.activation(r6s[:], r6[:], AF.Square)
            nc.vector.tensor_mul(u[:], r6s[:], r6[:])
            nc.scalar.activation(r7[:], xt[:], AF.Relu, bias=k0_t[:, 2:3])
            nc.scalar.activation(r7s[:], r7[:], AF.Square)
            nc.vector.tensor_mul(v[:], r7s[:], r7[:])
            nc.scalar.activation(sg[:], xt[:], AF.Sigmoid)
            nc.vector.tensor_mul(sl[:], sg[:], xt[:])
            feats = [xt, t2, t3, u, v, sl]

            # PSUM accumulation: one tile per (image-pair, row-block)
            ps = [[pp.tile([128, 2, WO], f32, name=f"ps{p}{b}", tag=f"ps{p}{b}")
                   for b in range(2)] for p in range(2)]
            for fi, ft in enumerate(feats):
                fr = ft[:, :].rearrange("p (i c) -> p i c", c=512)
                for dj in range(3):
                    for pair in range(2):
                        for blk in range(2):
                            idx = blk * 18 + fi * 3 + dj
                            nc.tensor.matmul(
                                ps[pair][blk][:, :, :],
                                bands_t[:, idx * 128:(idx + 1) * 128],
                                fr[:, 2 * pair:2 * pair + 2,
                                   blk * 256 + dj:blk * 256 + dj + WO],
                                start=(fi == 0 and dj == 0),
                                stop=(fi == 5 and dj == 2),
                            )

            # seam fix (rows 126/127 split across blocks) + export with +K0
            for pair in range(2):
                psA, psB = ps[pair]
                outa = pool.tile([128, 2, WO], f32, tag=f"oa{pair}")
                outb = pool.tile([128, 2, WO], f32, tag=f"ob{pair}")
                nc.scalar.activation(outa[:, :, :], psA[:, :, :],
                                     AF.Identity, bias=k0_t[:, 0:1])
                seam = pool.tile([2, 2, WO], f32, tag=f"seam{pair}")
                nc.sync.dma_start(seam[:, :, :], outa[126:128, :, :])
                nc.vector.tensor_add(psB[0:2, :, :], seam[:, :, :],
                                     psB[0:2, :, :])
                nc.scalar.activation(outb[:, :, :], psB[:, :, :],
                                     AF.Identity, bias=k0_t[:, 3:4])
                for q in range(2):
                    img = 2 * pair + q
                    nc.sync.dma_start(out_d[img, 0:126, :], outa[0:126, q, :])
                    nc.sync.dma_start(out_d[img, 126:254, :], outb[:, q, :])
    nc.compile()
    return nc


def kernel(x, base_weight, spline_weight):
    global _PROG
    x = np.ascontiguousarray(np.asarray(x, dtype=np.float32))
    bands, k0 = _build_weight_inputs(base_weight, spline_weight)
    if _PROG is None:
        _PROG = _build_program()
    in_maps = [{"x": x[c * BPC:(c + 1) * BPC], "bands": bands, "k0": k0}
               for c in range(NCORES)]
    res = run_bass_kernel_spmd(_PROG, in_maps, core_ids=list(range(NCORES)))
    return np.concatenate([res.results[c]["out"] for c in range(NCORES)], axis=0)
